# revision 14
# baseline (speedup 1.0000x reference)
"""LIF (leaky integrate-and-fire) spiking-neuron scan on 8 Trainium2 NeuronCores.

Reference semantics (per element, f32):
    h_t = v_{t-1} + (x_t - v_{t-1}) / 2        (tau = 2, v_reset = 0)
    s_t = (h_t >= 1)                           (spike, threshold v_th = 1)
    v_t = h_t * (1 - s_t)                      (hard reset)

Device formulation, tracking w_t = v_{t-1} + x_t (so h_t = w_t / 2):
    w_{t+1} = select(w_t < 2, 0.5 * w_t, 0) + x_{t+1}
computed by ONE custom DVE op per step (registered at runtime into
concourse's custom-DVE table; fp32, 1x mode, ~0.83us) instead of the
2x tensor_tensor + tensor_scalar chain (~2.2us/step).  The spike output
is produced on the otherwise-idle ACT engine in parallel:
    o_t = Sign(1 - 0.5 * w_t)  in  {-1, 0, +1}  -> fp8e4 (1 byte)
(sign-flipped so the bias const +1.0 is one bass pre-registers).  Host
decodes spike = (byte == 0xB8) (= -1.0 in e4m3; w_t == 2.0 exactly maps
to Sign(0) = 0 which we count as no-spike - measure-zero event).

Sharding: batch dim B=64 split across 8 cores (8 rows each); time stays
local (sequential scan).  DRAM layout is partition-major [128, T*512] so
every DMA segment is contiguous per partition.  All x loads ride the
sync HWDGE ring (sole user, FIFO at full HBM rate); output stores are
issued from the scalar queue right after the ACT op that fills them.
"""

import os
import numpy as np

T, B, N = 64, 64, 8192
NCORES = 8
BL = B // NCORES          # batch rows per core
P = 128                   # SBUF partitions
F = (BL * N) // P         # free elems per partition per step  (512)

# timestep chunking: small first chunks prime the pipeline, then steady-state
LOAD_CHUNKS = [4, 4, 4, 4] + [8] * 6
assert sum(LOAD_CHUNKS) == T
ST = 16                   # store granularity (timesteps)

SPIKE_BYTE = 0xB8         # -1.0 in float8_e4m3

_built = {}


def _register_lif_op():
    """Register the fused LIF-step custom DVE op:
        out = select(in0 < s0, in0 * s1, 0) + in1
    (s0 = 2.0 threshold on w, s1 = 0.5 leak).  Row/table generated at
    runtime through concourse's own lowering, sha self-consistent."""
    import concourse.dve_ops as dops
    if "lif_op" in _built:
        return _built["lif_op"]
    for op in dops.OPS:
        if op.name == "LIF_FUSED_STEP_ANT":
            _built["lif_op"] = op
            return op

    from concourse.dve_ops import DveOp
    from concourse.dve_spec import Spec, Src0, Src1, C0, C1, Zero, select, lower, _has_src1
    from concourse.dve_uop import DveOpSpec
    from concourse.dve_table_gen import dve_ver_for

    def _ref(in0, in1, s0, s1, imm2):
        a = in0.astype(np.float32)
        return (np.where(a < s0, a * s1, 0.0) + in1.astype(np.float32)).astype(
            np.float32
        )

    spec = Spec(body=select(Src0 < C0, Src0 * C1, Zero) + Src1, reference=_ref)
    row = dops._CUSTOM_DVE_ROW_BASE + len(dops.OPS)
    shas = {}
    for ver in ("v3", "v4"):
        try:
            sp = DveOpSpec(
                name="LIF_FUSED_STEP_ANT",
                opcode=row,
                uops=lower(spec, ver=ver),
                rd1_en=_has_src1(spec),
            )
            shas[ver] = sp.sha(ver)
        except Exception:
            pass
    op = DveOp("LIF_FUSED_STEP_ANT", spec, subdim=False, uops_sha=shas)
    dops.OPS.append(op)
    dops.CUSTOM_DVE_SPECS[op.name] = op.spec
    dops._SUB_OPCODE_FOR_NAME[op.name] = row
    _built["lif_op"] = op
    return op


def _build():
    if "nc" in _built:
        return _built["nc"]

    from contextlib import ExitStack
    import concourse.mybir as mybir
    from concourse import bacc, tile

    lif_op = _register_lif_op()

    # Slim the kernel-exit choreography: the stock exit is
    # drain -> all_engine_barrier -> clear sems -> all_engine_barrier; the
    # trailing barrier only orders the sem clears against later instructions,
    # of which there are none at kernel end (~3us saved).
    from concourse.vector_clock import ScopedClock

    def _slim_drain_and_barrier(self, tick_clock, wait_clock):
        drain_inst = self.nc.sync.drain()
        wait_clock.add_sem_waits(
            drain_inst.ins, ScopedClock({None: tick_clock.global_clock})
        )
        self.nc.all_engine_barrier()
        popped = self.nc._tile_sem_poison_stack.pop()
        assert popped is self._sem_poison
        self.nc.clear_and_free_semaphores(list(self.sems.allocated().values()))

    tile.TileContext._drain_and_barrier = _slim_drain_and_barrier

    nc = bacc.Bacc("TRN2", target_bir_lowering=False, debug=False)
    # partition-major layouts: [P, T*F] so per-partition bytes are contiguous
    x_ext = nc.dram_tensor("x", [P, T * F], mybir.dt.float32, kind="ExternalInput")
    m_ext = nc.dram_tensor("m", [P, T * F], mybir.dt.float8e4, kind="ExternalOutput")

    Sign = mybir.ActivationFunctionType.Sign

    WWIN = 56                 # w ring-buffer window (timesteps held in SBUF)
    SCALAR_LOADS = (1, 3)     # chunk indices loaded via the scalar HWDGE ring

    with tile.TileContext(nc) as tc:
        with ExitStack() as ctx:
            xp = ctx.enter_context(tc.tile_pool(name="xp", bufs=4))
            mp = ctx.enter_context(tc.tile_pool(name="mp", bufs=2))
            wp = ctx.enter_context(tc.tile_pool(name="wp", bufs=1))

            # issue all loads up front: most on the sync HWDGE ring, two
            # early chunks on the scalar ring so transfers run on both
            # rings in parallel and stay ahead of the DVE scan.  Tile
            # paces issues via the xp pool-buffer semaphores.
            x_tiles = []
            t0 = 0
            for ci, ch in enumerate(LOAD_CHUNKS):
                xt = xp.tile([P, ch * F], mybir.dt.float32, tag="xchunk",
                             name=f"xchunk{ci}")
                eng = nc.scalar if ci in SCALAR_LOADS else nc.sync
                eng.dma_start(out=xt[:], in_=x_ext[:, t0 * F:(t0 + ch) * F])
                x_tiles.append((t0, ch, xt))
                t0 += ch

            def x_slice(t):
                for (t0, ch, xt) in x_tiles:
                    if t0 <= t < t0 + ch:
                        return xt[:, (t - t0) * F:(t - t0 + 1) * F]
                raise AssertionError(t)

            # w state lives in ONE rolling [P, WWIN*F] tile (slice t%WWIN)
            # so DVE writers almost never carry pool-WAR semaphores, and
            # the ACT engine can read any contiguous group of steps.
            wbig = wp.tile([P, WWIN * F], mybir.dt.float32)

            def w_ap(t):
                return wbig[:, (t % WWIN) * F:(t % WWIN + 1) * F]

            def w_group(t0, n):  # steps t0..t0+n-1, contiguous mod WWIN
                a = t0 % WWIN
                assert a + n <= WWIN
                return wbig[:, a * F:(a + n) * F]

            nc.vector.tensor_copy(w_ap(0), x_slice(0))

            mt = None
            for t in range(T):
                if t % ST == 0:
                    mt = mp.tile([P, ST * F], mybir.dt.float8e4, tag="mchunk")
                # spike output on the ACT engine: o = Sign(1 - 0.5*w),
                # four steps per ACTIVATE (amortises the fixed cost); the
                # last four steps run as pair+singles so the final stores
                # don't wait on the whole last quad.
                if t < T - 4 and t % 4 == 3:
                    nc.scalar.activation(
                        mt[:, (t % ST - 3) * F:(t % ST + 1) * F],
                        w_group(t - 3, 4),
                        Sign,
                        bias=1.0,
                        scale=-0.5,
                    )
                elif t == T - 3:
                    nc.scalar.activation(
                        mt[:, (t % ST - 1) * F:(t % ST + 1) * F],
                        w_group(t - 1, 2),
                        Sign,
                        bias=1.0,
                        scale=-0.5,
                    )
                elif t >= T - 2:
                    nc.scalar.activation(
                        mt[:, (t % ST) * F:(t % ST + 1) * F],
                        w_ap(t),
                        Sign,
                        bias=1.0,
                        scale=-0.5,
                    )
                # fused state update on DVE: w' = select(w<2, 0.5w, 0) + x'
                if t < T - 1:
                    nc.vector._custom_dve(
                        lif_op,
                        out=w_ap(t + 1),
                        in0=w_ap(t),
                        in1=x_slice(t + 1),
                        s0=2.0,
                        s1=0.5,
                        imm2=0.0,
                    )
                # stores: issued from the scalar queue right after the ACT
                # op that filled the group (same-engine FIFO -> no waits);
                # final group split so the exit drain waits on less.
                if t == T - 3:
                    nc.scalar.dma_start(
                        out=m_ext[:, (t - (ST - 2) + 1) * F:(t + 1) * F],
                        in_=mt[:, :(ST - 2) * F],
                    )
                elif t == T - 1:
                    nc.sync.dma_start(
                        out=m_ext[:, (t - 1) * F:(t + 1) * F],
                        in_=mt[:, (ST - 2) * F:],
                    )
                elif t % ST == ST - 1:
                    nc.scalar.dma_start(
                        out=m_ext[:, (t - ST + 1) * F:(t + 1) * F],
                        in_=mt[:],
                    )

    nc.compile()
    _built["nc"] = nc
    return nc


def _install_ntff_hook() -> bool:
    """Provide antenv.axon_hooks (absent in this image) so that
    run_bass_kernel_spmd(trace=True) can capture NTFF profiles via the
    ctypes hook that trn_agent_boot already implements."""
    try:
        from antenv.axon_hooks import get_axon_ntff_profile_hook  # noqa: F401
        return True
    except ImportError:
        pass
    try:
        import sys
        import types
        import antenv
        from trn_agent_boot.trn_boot import _ntff_profile_via_ctypes

        hook = _ntff_profile_via_ctypes("/opt/axon/libaxon_pjrt.so")
        if hook is None:
            return False
        mod = types.ModuleType("antenv.axon_hooks")
        state = {"hook": hook}
        mod.get_axon_ntff_profile_hook = lambda: state["hook"]
        mod.set_axon_ntff_profile_hook = lambda h: state.__setitem__("hook", h)
        sys.modules["antenv.axon_hooks"] = mod
        antenv.axon_hooks = mod
        return True
    except Exception:
        return False


def kernel(x: np.ndarray) -> np.ndarray:
    import concourse.bass_utils as bass_utils

    nc = _build()

    x = np.asarray(x)
    assert x.shape == (T, B, N) and x.dtype == np.float32

    in_maps = []
    for c in range(NCORES):
        # [T, BL*N] -> [T, P, F] -> [P, T, F] -> [P, T*F]  (partition-major)
        shard = (
            x[:, c * BL:(c + 1) * BL, :]
            .reshape(T, P, F)
            .transpose(1, 0, 2)
            .reshape(P, T * F)
        )
        in_maps.append({"x": np.ascontiguousarray(shard)})

    trace = bool(int(os.environ.get("LIF_TRACE", "0")))
    if trace:
        trace = _install_ntff_hook()
        # artifact upload has no bucket in this container; neuter it
        bass_utils.upload_artifacts = lambda tmpdir: tmpdir

    try:
        res = bass_utils.run_bass_kernel_spmd(
            nc, in_maps, list(range(NCORES)), trace=trace
        )
    except Exception:
        if not trace:
            raise
        res = bass_utils.run_bass_kernel_spmd(
            nc, in_maps, list(range(NCORES)), trace=False
        )
    _built["last_result"] = res

    out = np.empty((T, B, N), np.float32)
    for c in range(NCORES):
        m = np.asarray(res.results[c]["m"])          # fp8e4 [P, T*F]
        bits = m.view(np.uint8).reshape(P, T, F).transpose(1, 0, 2)
        spikes = (bits == SPIKE_BYTE).astype(np.float32).reshape(T, BL, N)
        out[:, c * BL:(c + 1) * BL, :] = spikes
    return out


# revision 17
# speedup vs baseline: 1.0805x; 1.0805x over previous
"""LIF (leaky integrate-and-fire) spiking-neuron scan on 8 Trainium2 NeuronCores.

Reference semantics (per element, f32):
    h_t = v_{t-1} + (x_t - v_{t-1}) / 2        (tau = 2, v_reset = 0)
    s_t = (h_t >= 1)                           (spike, threshold v_th = 1)
    v_t = h_t * (1 - s_t)                      (hard reset)

Device formulation, tracking w_t = v_{t-1} + x_t (so h_t = w_t / 2):
    w_{t+1} = select(w_t < 2, 0.5 * w_t, 0) + x_{t+1}
computed by ONE custom DVE op per step (registered at runtime into
concourse's custom-DVE table; fp32, 1x mode, ~0.83us) instead of the
2x tensor_tensor + tensor_scalar chain (~2.2us/step).  The spike output
is produced on the otherwise-idle ACT engine in parallel:
    o_t = Sign(1 - 0.5 * w_t)  in  {-1, 0, +1}  -> fp8e4 (1 byte)
(sign-flipped so the bias const +1.0 is one bass pre-registers).  Host
decodes spike = (byte == 0xB8) (= -1.0 in e4m3; w_t == 2.0 exactly maps
to Sign(0) = 0 which we count as no-spike - measure-zero event).

Sharding: batch dim B=64 split across 8 cores (8 rows each); time stays
local (sequential scan).  DRAM layout is partition-major [128, T*512] so
every DMA segment is contiguous per partition.  All x loads ride the
sync HWDGE ring (sole user, FIFO at full HBM rate); output stores are
issued from the scalar queue right after the ACT op that fills them.
"""

import os
import numpy as np

T, B, N = 64, 64, 8192
NCORES = 8
BL = B // NCORES          # batch rows per core
P = 128                   # SBUF partitions
F = (BL * N) // P         # free elems per partition per step  (512)

# timestep chunking: small first chunks prime the pipeline, then steady-state
LOAD_CHUNKS = [4, 4] + [8] * 7
assert sum(LOAD_CHUNKS) == T
ST = 16                   # store granularity (timesteps)

SPIKE_BYTE = 0xB8         # -1.0 in float8_e4m3

_built = {}


def _register_lif_op():
    """Register the fused LIF-step custom DVE op:
        out = select(in0 < s0, in0 * s1, 0) + in1
    (s0 = 2.0 threshold on w, s1 = 0.5 leak).  Row/table generated at
    runtime through concourse's own lowering, sha self-consistent."""
    import concourse.dve_ops as dops
    if "lif_op" in _built:
        return _built["lif_op"]
    for op in dops.OPS:
        if op.name == "LIF_FUSED_STEP_ANT":
            _built["lif_op"] = op
            return op

    from concourse.dve_ops import DveOp
    from concourse.dve_spec import Spec, Src0, Src1, C0, C1, Zero, select, lower, _has_src1
    from concourse.dve_uop import DveOpSpec
    from concourse.dve_table_gen import dve_ver_for

    def _ref(in0, in1, s0, s1, imm2):
        a = in0.astype(np.float32)
        return (np.where(a < s0, a * s1, 0.0) + in1.astype(np.float32)).astype(
            np.float32
        )

    spec = Spec(body=select(Src0 < C0, Src0 * C1, Zero) + Src1, reference=_ref)
    row = dops._CUSTOM_DVE_ROW_BASE + len(dops.OPS)
    shas = {}
    for ver in ("v3", "v4"):
        try:
            sp = DveOpSpec(
                name="LIF_FUSED_STEP_ANT",
                opcode=row,
                uops=lower(spec, ver=ver),
                rd1_en=_has_src1(spec),
            )
            shas[ver] = sp.sha(ver)
        except Exception:
            pass
    op = DveOp("LIF_FUSED_STEP_ANT", spec, subdim=False, uops_sha=shas)
    dops.OPS.append(op)
    dops.CUSTOM_DVE_SPECS[op.name] = op.spec
    dops._SUB_OPCODE_FOR_NAME[op.name] = row
    _built["lif_op"] = op
    return op


def _build():
    if "nc" in _built:
        return _built["nc"]

    from contextlib import ExitStack
    import concourse.mybir as mybir
    from concourse import bacc, tile

    lif_op = _register_lif_op()

    # Slim the kernel-exit choreography: the stock exit is
    # drain -> all_engine_barrier -> clear sems -> all_engine_barrier; the
    # trailing barrier only orders the sem clears against later instructions,
    # of which there are none at kernel end (~3us saved).
    from concourse.vector_clock import ScopedClock

    def _slim_drain_and_barrier(self, tick_clock, wait_clock):
        drain_inst = self.nc.sync.drain()
        wait_clock.add_sem_waits(
            drain_inst.ins, ScopedClock({None: tick_clock.global_clock})
        )
        self.nc.all_engine_barrier()
        popped = self.nc._tile_sem_poison_stack.pop()
        assert popped is self._sem_poison
        self.nc.clear_and_free_semaphores(list(self.sems.allocated().values()))

    tile.TileContext._drain_and_barrier = _slim_drain_and_barrier

    nc = bacc.Bacc("TRN2", target_bir_lowering=False, debug=False)
    # partition-major layouts: [P, T*F] so per-partition bytes are contiguous
    x_ext = nc.dram_tensor("x", [P, T * F], mybir.dt.float32, kind="ExternalInput")
    m_ext = nc.dram_tensor("m", [P, T * F], mybir.dt.float8e4, kind="ExternalOutput")

    Sign = mybir.ActivationFunctionType.Sign

    SCALAR_LOADS = (1, 3)     # chunk indices loaded via the scalar HWDGE ring

    with tile.TileContext(nc) as tc:
        with ExitStack() as ctx:
            xp = ctx.enter_context(tc.tile_pool(name="xp", bufs=7))
            mp = ctx.enter_context(tc.tile_pool(name="mp", bufs=2))
            wp = ctx.enter_context(tc.tile_pool(name="wp", bufs=4))

            # issue all loads up front: most on the sync HWDGE ring, two
            # early chunks on the scalar ring so transfers run on both
            # rings in parallel and stay ahead of the DVE scan.  Tile
            # paces issues via the xp pool-buffer semaphores.
            x_tiles = []
            t0 = 0
            for ci, ch in enumerate(LOAD_CHUNKS):
                xt = xp.tile([P, ch * F], mybir.dt.float32, tag="xchunk",
                             name=f"xchunk{ci}")
                eng = nc.scalar if ci in SCALAR_LOADS else nc.sync
                eng.dma_start(out=xt[:], in_=x_ext[:, t0 * F:(t0 + ch) * F])
                x_tiles.append((t0, ch, xt))
                t0 += ch

            def x_slice(t):
                for (t0, ch, xt) in x_tiles:
                    if t0 <= t < t0 + ch:
                        return xt[:, (t - t0) * F:(t - t0 + 1) * F]
                raise AssertionError(t)

            # w state lives in QUAD tiles [P, 4F] holding 4 consecutive
            # steps, so the ACT engine emits spikes for four steps per
            # ACTIVATE while DVE pool-WAR waits stay 16 steps behind.
            quads = [wp.tile([P, 4 * F], mybir.dt.float32, tag="wquad",
                             name=f"wquad{q}")
                     for q in range(T // 4)]

            def w_ap(t):
                return quads[t // 4][:, (t % 4) * F:(t % 4 + 1) * F]

            def w_group(t0, n):  # steps t0..t0+n-1 within one quad
                q, a = t0 // 4, t0 % 4
                assert a + n <= 4
                return quads[q][:, a * F:(a + n) * F]

            nc.vector.tensor_copy(w_ap(0), x_slice(0))

            mt = None
            for t in range(T):
                if t % ST == 0:
                    mt = mp.tile([P, ST * F], mybir.dt.float8e4, tag="mchunk")
                # spike output on the ACT engine: o = Sign(1 - 0.5*w),
                # four steps per ACTIVATE (amortises the fixed cost); the
                # last four steps run as pair+singles so the final stores
                # don't wait on the whole last quad.
                if t < T - 4 and t % 4 == 3:
                    nc.scalar.activation(
                        mt[:, (t % ST - 3) * F:(t % ST + 1) * F],
                        w_group(t - 3, 4),
                        Sign,
                        bias=1.0,
                        scale=-0.5,
                    )
                elif t == T - 3:
                    nc.scalar.activation(
                        mt[:, (t % ST - 1) * F:(t % ST + 1) * F],
                        w_group(t - 1, 2),
                        Sign,
                        bias=1.0,
                        scale=-0.5,
                    )
                elif t >= T - 2:
                    nc.scalar.activation(
                        mt[:, (t % ST) * F:(t % ST + 1) * F],
                        w_ap(t),
                        Sign,
                        bias=1.0,
                        scale=-0.5,
                    )
                # fused state update on DVE: w' = select(w<2, 0.5w, 0) + x'
                if t < T - 1:
                    nc.vector._custom_dve(
                        lif_op,
                        out=w_ap(t + 1),
                        in0=w_ap(t),
                        in1=x_slice(t + 1),
                        s0=2.0,
                        s1=0.5,
                        imm2=0.0,
                    )
                # stores: issued from the scalar queue right after the ACT
                # op that filled the group (same-engine FIFO -> no waits);
                # final group split so the exit drain waits on less.
                if t == T - 3:
                    nc.scalar.dma_start(
                        out=m_ext[:, (t - (ST - 2) + 1) * F:(t + 1) * F],
                        in_=mt[:, :(ST - 2) * F],
                    )
                elif t == T - 1:
                    nc.sync.dma_start(
                        out=m_ext[:, (t - 1) * F:(t + 1) * F],
                        in_=mt[:, (ST - 2) * F:],
                    )
                elif t % ST == ST - 1:
                    nc.scalar.dma_start(
                        out=m_ext[:, (t - ST + 1) * F:(t + 1) * F],
                        in_=mt[:],
                    )

    nc.compile()
    _built["nc"] = nc
    return nc


def _install_ntff_hook() -> bool:
    """Provide antenv.axon_hooks (absent in this image) so that
    run_bass_kernel_spmd(trace=True) can capture NTFF profiles via the
    ctypes hook that trn_agent_boot already implements."""
    try:
        from antenv.axon_hooks import get_axon_ntff_profile_hook  # noqa: F401
        return True
    except ImportError:
        pass
    try:
        import sys
        import types
        import antenv
        from trn_agent_boot.trn_boot import _ntff_profile_via_ctypes

        hook = _ntff_profile_via_ctypes("/opt/axon/libaxon_pjrt.so")
        if hook is None:
            return False
        mod = types.ModuleType("antenv.axon_hooks")
        state = {"hook": hook}
        mod.get_axon_ntff_profile_hook = lambda: state["hook"]
        mod.set_axon_ntff_profile_hook = lambda h: state.__setitem__("hook", h)
        sys.modules["antenv.axon_hooks"] = mod
        antenv.axon_hooks = mod
        return True
    except Exception:
        return False


def kernel(x: np.ndarray) -> np.ndarray:
    import concourse.bass_utils as bass_utils

    nc = _build()

    x = np.asarray(x)
    assert x.shape == (T, B, N) and x.dtype == np.float32

    in_maps = []
    for c in range(NCORES):
        # [T, BL*N] -> [T, P, F] -> [P, T, F] -> [P, T*F]  (partition-major)
        shard = (
            x[:, c * BL:(c + 1) * BL, :]
            .reshape(T, P, F)
            .transpose(1, 0, 2)
            .reshape(P, T * F)
        )
        in_maps.append({"x": np.ascontiguousarray(shard)})

    trace = bool(int(os.environ.get("LIF_TRACE", "0")))
    if trace:
        trace = _install_ntff_hook()
        # artifact upload has no bucket in this container; neuter it
        bass_utils.upload_artifacts = lambda tmpdir: tmpdir

    try:
        res = bass_utils.run_bass_kernel_spmd(
            nc, in_maps, list(range(NCORES)), trace=trace
        )
    except Exception:
        if not trace:
            raise
        res = bass_utils.run_bass_kernel_spmd(
            nc, in_maps, list(range(NCORES)), trace=False
        )
    _built["last_result"] = res

    out = np.empty((T, B, N), np.float32)
    for c in range(NCORES):
        m = np.asarray(res.results[c]["m"])          # fp8e4 [P, T*F]
        bits = m.view(np.uint8).reshape(P, T, F).transpose(1, 0, 2)
        spikes = (bits == SPIKE_BYTE).astype(np.float32).reshape(T, BL, N)
        out[:, c * BL:(c + 1) * BL, :] = spikes
    return out


# revision 19
# speedup vs baseline: 1.1330x; 1.0486x over previous
"""LIF (leaky integrate-and-fire) spiking-neuron scan on 8 Trainium2 NeuronCores.

Reference semantics (per element, f32):
    h_t = v_{t-1} + (x_t - v_{t-1}) / 2        (tau = 2, v_reset = 0)
    s_t = (h_t >= 1)                           (spike, threshold v_th = 1)
    v_t = h_t * (1 - s_t)                      (hard reset)

Device formulation, tracking w_t = v_{t-1} + x_t (so h_t = w_t / 2):
    w_{t+1} = select(w_t < 2, 0.5 * w_t, 0) + x_{t+1}
computed by ONE custom DVE op per step (registered at runtime into
concourse's custom-DVE table; fp32, 1x mode, ~0.83us) instead of the
2x tensor_tensor + tensor_scalar chain (~2.2us/step).  The spike output
is produced on the otherwise-idle ACT engine in parallel:
    o_t = Sign(1 - 0.5 * w_t)  in  {-1, 0, +1}  -> fp8e4 (1 byte)
(sign-flipped so the bias const +1.0 is one bass pre-registers).  Host
decodes spike = (byte == 0xB8) (= -1.0 in e4m3; w_t == 2.0 exactly maps
to Sign(0) = 0 which we count as no-spike - measure-zero event).

Sharding: batch dim B=64 split across 8 cores (8 rows each); time stays
local (sequential scan).  DRAM layout is partition-major [128, T*512] so
every DMA segment is contiguous per partition.  All x loads ride the
sync HWDGE ring (sole user, FIFO at full HBM rate); output stores are
issued from the scalar queue right after the ACT op that fills them.
"""

import os
import numpy as np

T, B, N = 64, 64, 8192
NCORES = 8
BL = B // NCORES          # batch rows per core
P = 128                   # SBUF partitions
F = (BL * N) // P         # free elems per partition per step  (512)

# timestep chunking: small first chunks prime the pipeline, then steady-state
LOAD_CHUNKS = [4, 4] + [8] * 6 + [4, 2, 2]
assert sum(LOAD_CHUNKS) == T
ST = 16                   # store granularity (timesteps)

SPIKE_BYTE = 0xB8         # -1.0 in float8_e4m3

_built = {}


def _register_lif_op():
    """Register the fused LIF-step custom DVE op:
        out = select(in0 < s0, in0 * s1, 0) + in1
    (s0 = 2.0 threshold on w, s1 = 0.5 leak).  Row/table generated at
    runtime through concourse's own lowering, sha self-consistent."""
    import concourse.dve_ops as dops
    if "lif_op" in _built:
        return _built["lif_op"]
    for op in dops.OPS:
        if op.name == "LIF_FUSED_STEP_ANT":
            _built["lif_op"] = op
            return op

    from concourse.dve_ops import DveOp
    from concourse.dve_spec import Spec, Src0, Src1, C0, C1, Zero, select, lower, _has_src1
    from concourse.dve_uop import DveOpSpec
    from concourse.dve_table_gen import dve_ver_for

    def _ref(in0, in1, s0, s1, imm2):
        a = in0.astype(np.float32)
        return (np.where(a < s0, a * s1, 0.0) + in1.astype(np.float32)).astype(
            np.float32
        )

    spec = Spec(body=select(Src0 < C0, Src0 * C1, Zero) + Src1, reference=_ref)
    row = dops._CUSTOM_DVE_ROW_BASE + len(dops.OPS)
    shas = {}
    for ver in ("v3", "v4"):
        try:
            sp = DveOpSpec(
                name="LIF_FUSED_STEP_ANT",
                opcode=row,
                uops=lower(spec, ver=ver),
                rd1_en=_has_src1(spec),
            )
            shas[ver] = sp.sha(ver)
        except Exception:
            pass
    op = DveOp("LIF_FUSED_STEP_ANT", spec, subdim=False, uops_sha=shas)
    dops.OPS.append(op)
    dops.CUSTOM_DVE_SPECS[op.name] = op.spec
    dops._SUB_OPCODE_FOR_NAME[op.name] = row
    _built["lif_op"] = op
    return op


def _build():
    if "nc" in _built:
        return _built["nc"]

    from contextlib import ExitStack
    import concourse.mybir as mybir
    from concourse import bacc, tile

    lif_op = _register_lif_op()

    # Slim the kernel-exit choreography: the stock exit is
    # drain -> all_engine_barrier -> clear sems -> all_engine_barrier; the
    # trailing barrier only orders the sem clears against later instructions,
    # of which there are none at kernel end (~3us saved).
    from concourse.vector_clock import ScopedClock

    def _slim_drain_and_barrier(self, tick_clock, wait_clock):
        drain_inst = self.nc.sync.drain()
        wait_clock.add_sem_waits(
            drain_inst.ins, ScopedClock({None: tick_clock.global_clock})
        )
        self.nc.all_engine_barrier()
        popped = self.nc._tile_sem_poison_stack.pop()
        assert popped is self._sem_poison
        self.nc.clear_and_free_semaphores(list(self.sems.allocated().values()))

    tile.TileContext._drain_and_barrier = _slim_drain_and_barrier

    nc = bacc.Bacc("TRN2", target_bir_lowering=False, debug=False)
    # partition-major layouts: [P, T*F] so per-partition bytes are contiguous
    x_ext = nc.dram_tensor("x", [P, T * F], mybir.dt.float32, kind="ExternalInput")
    m_ext = nc.dram_tensor("m", [P, T * F], mybir.dt.float8e4, kind="ExternalOutput")

    Sign = mybir.ActivationFunctionType.Sign

    SCALAR_LOADS = ()         # chunk indices loaded via the scalar HWDGE ring
                              # (both rings share the 16 SDMA engines, so
                              # splitting only reorders service -- keep all
                              # loads on sync in consumption order)

    with tile.TileContext(nc) as tc:
        with ExitStack() as ctx:
            xp = ctx.enter_context(tc.tile_pool(name="xp", bufs=8))
            mp = ctx.enter_context(tc.tile_pool(name="mp", bufs=2))
            wp = ctx.enter_context(tc.tile_pool(name="wp", bufs=6))

            # issue all loads up front: most on the sync HWDGE ring, two
            # early chunks on the scalar ring so transfers run on both
            # rings in parallel and stay ahead of the DVE scan.  Tile
            # paces issues via the xp pool-buffer semaphores.
            x_tiles = []
            t0 = 0
            for ci, ch in enumerate(LOAD_CHUNKS):
                xt = xp.tile([P, ch * F], mybir.dt.float32, tag="xchunk",
                             name=f"xchunk{ci}")
                eng = nc.scalar if ci in SCALAR_LOADS else nc.sync
                eng.dma_start(out=xt[:], in_=x_ext[:, t0 * F:(t0 + ch) * F])
                x_tiles.append((t0, ch, xt))
                t0 += ch

            def x_slice(t):
                for (t0, ch, xt) in x_tiles:
                    if t0 <= t < t0 + ch:
                        return xt[:, (t - t0) * F:(t - t0 + 1) * F]
                raise AssertionError(t)

            # w state lives in QUAD tiles [P, 4F] holding 4 consecutive
            # steps, so the ACT engine emits spikes for four steps per
            # ACTIVATE while DVE pool-WAR waits stay 16 steps behind.
            quads = [wp.tile([P, 4 * F], mybir.dt.float32, tag="wquad",
                             name=f"wquad{q}")
                     for q in range(T // 4)]

            def w_ap(t):
                return quads[t // 4][:, (t % 4) * F:(t % 4 + 1) * F]

            def w_group(t0, n):  # steps t0..t0+n-1 within one quad
                q, a = t0 // 4, t0 % 4
                assert a + n <= 4
                return quads[q][:, a * F:(a + n) * F]

            nc.vector.tensor_copy(w_ap(0), x_slice(0))

            mt = None
            for t in range(T):
                if t % ST == 0:
                    mt = mp.tile([P, ST * F], mybir.dt.float8e4, tag="mchunk")
                # spike output on the ACT engine: o = Sign(1 - 0.5*w),
                # four steps per ACTIVATE (amortises the fixed cost); the
                # last four steps run as pair+singles so the final stores
                # don't wait on the whole last quad.
                if t < T - 4 and t % 4 == 3:
                    nc.scalar.activation(
                        mt[:, (t % ST - 3) * F:(t % ST + 1) * F],
                        w_group(t - 3, 4),
                        Sign,
                        bias=1.0,
                        scale=-0.5,
                    )
                elif t == T - 3:
                    nc.scalar.activation(
                        mt[:, (t % ST - 1) * F:(t % ST + 1) * F],
                        w_group(t - 1, 2),
                        Sign,
                        bias=1.0,
                        scale=-0.5,
                    )
                elif t >= T - 2:
                    nc.scalar.activation(
                        mt[:, (t % ST) * F:(t % ST + 1) * F],
                        w_ap(t),
                        Sign,
                        bias=1.0,
                        scale=-0.5,
                    )
                # fused state update on DVE: w' = select(w<2, 0.5w, 0) + x'
                if t < T - 1:
                    nc.vector._custom_dve(
                        lif_op,
                        out=w_ap(t + 1),
                        in0=w_ap(t),
                        in1=x_slice(t + 1),
                        s0=2.0,
                        s1=0.5,
                        imm2=0.0,
                    )
                # stores: issued from the scalar queue right after the ACT
                # op that filled the group (same-engine FIFO -> no waits);
                # final group split so the exit drain waits on less.
                if t == T - 3:
                    nc.scalar.dma_start(
                        out=m_ext[:, (t - (ST - 2) + 1) * F:(t + 1) * F],
                        in_=mt[:, :(ST - 2) * F],
                    )
                elif t == T - 1:
                    nc.sync.dma_start(
                        out=m_ext[:, (t - 1) * F:(t + 1) * F],
                        in_=mt[:, (ST - 2) * F:],
                    )
                elif t % ST == ST - 1:
                    nc.scalar.dma_start(
                        out=m_ext[:, (t - ST + 1) * F:(t + 1) * F],
                        in_=mt[:],
                    )

    nc.compile()
    _built["nc"] = nc
    return nc


def _install_ntff_hook() -> bool:
    """Provide antenv.axon_hooks (absent in this image) so that
    run_bass_kernel_spmd(trace=True) can capture NTFF profiles via the
    ctypes hook that trn_agent_boot already implements."""
    try:
        from antenv.axon_hooks import get_axon_ntff_profile_hook  # noqa: F401
        return True
    except ImportError:
        pass
    try:
        import sys
        import types
        import antenv
        from trn_agent_boot.trn_boot import _ntff_profile_via_ctypes

        hook = _ntff_profile_via_ctypes("/opt/axon/libaxon_pjrt.so")
        if hook is None:
            return False
        mod = types.ModuleType("antenv.axon_hooks")
        state = {"hook": hook}
        mod.get_axon_ntff_profile_hook = lambda: state["hook"]
        mod.set_axon_ntff_profile_hook = lambda h: state.__setitem__("hook", h)
        sys.modules["antenv.axon_hooks"] = mod
        antenv.axon_hooks = mod
        return True
    except Exception:
        return False


def kernel(x: np.ndarray) -> np.ndarray:
    import concourse.bass_utils as bass_utils

    nc = _build()

    x = np.asarray(x)
    assert x.shape == (T, B, N) and x.dtype == np.float32

    in_maps = []
    for c in range(NCORES):
        # [T, BL*N] -> [T, P, F] -> [P, T, F] -> [P, T*F]  (partition-major)
        shard = (
            x[:, c * BL:(c + 1) * BL, :]
            .reshape(T, P, F)
            .transpose(1, 0, 2)
            .reshape(P, T * F)
        )
        in_maps.append({"x": np.ascontiguousarray(shard)})

    trace = bool(int(os.environ.get("LIF_TRACE", "0")))
    if trace:
        trace = _install_ntff_hook()
        # artifact upload has no bucket in this container; neuter it
        bass_utils.upload_artifacts = lambda tmpdir: tmpdir

    try:
        res = bass_utils.run_bass_kernel_spmd(
            nc, in_maps, list(range(NCORES)), trace=trace
        )
    except Exception:
        if not trace:
            raise
        res = bass_utils.run_bass_kernel_spmd(
            nc, in_maps, list(range(NCORES)), trace=False
        )
    _built["last_result"] = res

    out = np.empty((T, B, N), np.float32)
    for c in range(NCORES):
        m = np.asarray(res.results[c]["m"])          # fp8e4 [P, T*F]
        bits = m.view(np.uint8).reshape(P, T, F).transpose(1, 0, 2)
        spikes = (bits == SPIKE_BYTE).astype(np.float32).reshape(T, BL, N)
        out[:, c * BL:(c + 1) * BL, :] = spikes
    return out
